# revision 1
# baseline (speedup 1.0000x reference)
"""Single-head causal self-attention on 8 Trainium2 NeuronCores (Bass/Tile).

Problem: x [1024, 256, 384], Wq/Wk/Wv [384, 64] ->
  q,k,v = x@W;  wei = softmax(mask(q k^T / sqrt(384)));  out = wei @ v
Output: [1024, 256, 64] fp32.

Strategy (data-parallel over batch, 128 batches per core):
  - Host pre-transposes x to xT[b, p, c, t] = x[b, t, 128c+p] so the
    contraction dim (C=384, in 3 chunks of 128) lands on SBUF partitions
    with fully contiguous 1KB DMA rows.
  - Per batch, all matmuls run in fp32r (1 cycle/row when moving dim >= 256):
      qk   [128,256] = [Wq|Wk]^T x^T        (3-chunk accumulation)
      vT   [64,256]  = Wv^T x^T             (3-chunk accumulation)
      v    [128,128] = PE-transpose(vT)     (two 64x128 transposes)
      weiT [s,t]     = k q^T                (2 s-halves, K=64)
      P    = exp(weiT/sqrt(384)) * causal   (no max-subtraction: |wei/19.6|<~3)
      outT [65,256]  = [1|v]^T P            (ones col -> row 0 = softmax denom)
      out  = outT[1:65] * broadcast(1/denom)  (broadcast via K=1 matmul)
  - Causal structure: s-half0 is fully valid for t>=128 (mask only the
    diagonal 128x128 block); s-half1 is all-invalid for t<128 (left half of
    P1 kept at a persistent 0), diag-masked for t>=128.
  - Output written as outT [b, h, t]; host transposes back to [b, t, h].
"""

import os
from contextlib import ExitStack

import numpy as np

import concourse.bass as bass
import concourse.bacc as bacc
import concourse.tile as tile
from concourse import mybir
from concourse.bass_utils import run_bass_kernel_spmd

N_CORES = 8
B = 1024
T = 256
C = 384
H = 64
BPC = B // N_CORES  # 128 batches per core
NCHUNK = C // 128  # 3
SCALE = float(C) ** -0.5

F32 = mybir.dt.float32
F32R = mybir.dt.float32r


def r(ap):
    """Bitcast an fp32 AP to fp32r for full-rate matmul streaming."""
    return ap.bitcast(F32R)


def build_nc(bpc: int = BPC):
    nc = bacc.Bacc(
        "TRN2", target_bir_lowering=False, debug=False, num_devices=N_CORES
    )

    xT = nc.dram_tensor("xT", [bpc, 128, NCHUNK, T], F32R, kind="ExternalInput").ap()
    wqk = nc.dram_tensor("wqk", [128, NCHUNK, 128], F32R, kind="ExternalInput").ap()
    wv = nc.dram_tensor("wv", [128, NCHUNK, H], F32R, kind="ExternalInput").ap()
    mask = nc.dram_tensor("mask", [128, 128], F32, kind="ExternalInput").ap()
    eye = nc.dram_tensor("eye", [H, H], F32R, kind="ExternalInput").ap()
    ones = nc.dram_tensor("ones", [1, H], F32R, kind="ExternalInput").ap()
    outT = nc.dram_tensor("outT", [bpc, H, T], F32, kind="ExternalOutput").ap()

    with ExitStack() as ctx:
        tc = ctx.enter_context(tile.TileContext(nc))

        const = ctx.enter_context(tc.tile_pool(name="const", bufs=1))
        wqk_sb = const.tile([128, NCHUNK, 128], F32R, tag="wqk")
        nc.sync.dma_start(wqk_sb[:], wqk)
        wv_sb = const.tile([128, NCHUNK, H], F32R, tag="wv")
        nc.sync.dma_start(wv_sb[:], wv)
        mask_sb = const.tile([128, 128], F32, tag="mask")
        nc.sync.dma_start(mask_sb[:], mask)
        eye_sb = const.tile([H, H], F32R, tag="eye")
        nc.sync.dma_start(eye_sb[:], eye)
        ones_sb = const.tile([1, H], F32R, tag="ones")
        nc.sync.dma_start(ones_sb[:], ones)

        # Persistent double-buffered tiles with preset regions that survive
        # across iterations: v_aug ones-columns (0 and 65) and P1's zero
        # left half (the all-invalid causal block).
        NSLOT = 2
        vaug = []
        p1s = []
        for i in range(NSLOT):
            v_t = const.tile([128, 131], F32R, tag=f"vaug{i}")
            nc.gpsimd.memset(v_t[:, 64:65].bitcast(F32), 1.0)
            nc.gpsimd.memset(v_t[:, 129:130].bitcast(F32), 1.0)
            vaug.append(v_t)
            p_t = const.tile([128, T], F32R, tag=f"p1_{i}")
            nc.gpsimd.memset(p_t[:, 0:128].bitcast(F32), 0.0)
            p1s.append(p_t)

        xt_pool = ctx.enter_context(tc.tile_pool(name="xt", bufs=4))
        sb_pool = ctx.enter_context(tc.tile_pool(name="sb", bufs=2))
        psa_pool = ctx.enter_context(tc.tile_pool(name="psa", bufs=2, space="PSUM"))
        psb_pool = ctx.enter_context(tc.tile_pool(name="psb", bufs=2, space="PSUM"))
        psc_pool = ctx.enter_context(tc.tile_pool(name="psc", bufs=2, space="PSUM"))
        psd_pool = ctx.enter_context(tc.tile_pool(name="psd", bufs=2, space="PSUM"))

        for b in range(bpc):
            slot = b % NSLOT
            v_sb = vaug[slot]
            p1 = p1s[slot]

            xt = xt_pool.tile([128, NCHUNK, T], F32R, tag="xt")
            nc.sync.dma_start(xt[:], xT[b])

            # qk^T [128, 256] (q heads on partitions 0:64, k heads 64:128)
            # and v^T [64, 256], both accumulated over the 3 C-chunks.
            ps_a = psa_pool.tile([128, 512], F32, tag="psa")
            for c in range(NCHUNK):
                nc.tensor.matmul(
                    ps_a[:, 0:T],
                    lhsT=r(wqk_sb[:, c, :]),
                    rhs=r(xt[:, c, :]),
                    start=(c == 0),
                    stop=(c == NCHUNK - 1),
                )
            for c in range(NCHUNK):
                nc.tensor.matmul(
                    ps_a[0:H, T : T + T],
                    lhsT=r(wv_sb[:, c, :]),
                    rhs=r(xt[:, c, :]),
                    start=(c == 0),
                    stop=(c == NCHUNK - 1),
                )

            # q/k copied to separate base-0 tiles (matmul requires lhsT and
            # rhs at the same SBUF base partition).
            q_sb = sb_pool.tile([H, T], F32R, tag="q")
            nc.scalar.copy(q_sb[:], ps_a[0:H, 0:T])
            k_sb = sb_pool.tile([H, T], F32R, tag="k")
            nc.scalar.copy(k_sb[:], ps_a[H:128, 0:T])
            vt_sb = sb_pool.tile([H, T], F32R, tag="vt")
            nc.scalar.copy(vt_sb[:], ps_a[0:H, T : T + T])

            # v [s, h] via two PE transposes of vT s-halves.
            ps_b = psb_pool.tile([128, 128], F32, tag="psb")
            nc.tensor.transpose(r(ps_b[:, 0:64]), r(vt_sb[:, 0:128]), r(eye_sb[:]))
            nc.tensor.transpose(r(ps_b[:, 64:128]), r(vt_sb[:, 128:256]), r(eye_sb[:]))
            # One strided copy drops both halves into v_aug at cols 0:64 and
            # 65:129 (cols 64 and 129 hold the persistent ones).
            dst = v_sb[:, 0:130].rearrange("p (two f) -> p two f", two=2)[:, :, 0:64]
            src = ps_b[:, 0:128].rearrange("p (two f) -> p two f", two=2)
            nc.vector.tensor_copy(dst, src)

            # weiT[s, t] = k q^T for both s-halves (K = 64 heads).
            ps_c = psc_pool.tile([128, 512], F32, tag="psc")
            nc.tensor.matmul(
                ps_c[:, 0:T],
                lhsT=r(k_sb[:, 0:128]),
                rhs=r(q_sb[:]),
                start=True,
                stop=True,
            )
            nc.tensor.matmul(
                ps_c[:, T : T + T],
                lhsT=r(k_sb[:, 128:256]),
                rhs=r(q_sb[:]),
                start=True,
                stop=True,
            )

            # P = exp(weiT * scale); no max-subtraction needed (|arg| < ~3).
            p0 = sb_pool.tile([128, T], F32R, tag="p0")
            nc.scalar.activation(
                p0[:], ps_c[:, 0:T], mybir.ActivationFunctionType.Exp, scale=SCALE
            )
            nc.scalar.activation(
                p1[:, 128:256],
                ps_c[:, T + 128 : T + 256],
                mybir.ActivationFunctionType.Exp,
                scale=SCALE,
            )
            # Causal mask on the two diagonal blocks (GPSIMD, off DVE/ACT).
            nc.gpsimd.tensor_mul(p0[:, 0:128], p0[:, 0:128], mask_sb[:])
            nc.gpsimd.tensor_mul(p1[:, 128:256], p1[:, 128:256], mask_sb[:])

            # outT[65, 256]: row 64 = softmax denominator (ones columns),
            # rows 0:64 = unnormalized out^T. Accumulate both s-halves.
            ps_d = psd_pool.tile([128, 512], F32, tag="psd")
            nc.tensor.matmul(
                ps_d[0:65, 0:T],
                lhsT=r(v_sb[:, 0:65]),
                rhs=r(p0[:]),
                start=True,
                stop=False,
            )
            nc.tensor.matmul(
                ps_d[0:65, 0:T],
                lhsT=r(v_sb[:, 65:130]),
                rhs=r(p1[:]),
                start=False,
                stop=True,
            )

            recip = sb_pool.tile([1, T], F32R, tag="recip")
            with nc.allow_low_precision(reason="softmax denom reciprocal to f32r"):
                nc.vector.reciprocal(recip[:], ps_d[64:65, 0:T])
            # Broadcast 1/denom across 64 partitions via K=1 matmul.
            nc.tensor.matmul(
                ps_d[0:H, T : T + T],
                lhsT=r(ones_sb[:]),
                rhs=r(recip[:]),
                start=True,
                stop=True,
            )
            bc_sb = sb_pool.tile([H, T], F32, tag="bc")
            nc.scalar.copy(bc_sb[:], ps_d[0:H, T : T + T])
            out_sb = sb_pool.tile([H, T], F32, tag="out")
            nc.vector.tensor_mul(out_sb[:], ps_d[0:H, 0:T], bc_sb[:])
            nc.gpsimd.dma_start(outT[b], out_sb[:])

    nc.finalize()  # run Bacc passes (reg alloc, wait splitting) for BIR export
    return nc


def _host_inputs(x, Wq, Wk, Wv):
    B_, T_, C_ = x.shape
    assert (B_, T_, C_) == (B, T, C), (B_, T_, C_)
    xh = np.ascontiguousarray(
        x.reshape(B, T, NCHUNK, 128).transpose(0, 3, 2, 1), dtype=np.float32
    )  # [B, 128, 3, T];  xh[b, p, c, t] == x[b, t, 128c+p]
    wqk_h = np.ascontiguousarray(
        np.concatenate([Wq, Wk], axis=1).reshape(NCHUNK, 128, 128).transpose(1, 0, 2),
        dtype=np.float32,
    )
    wv_h = np.ascontiguousarray(
        Wv.reshape(NCHUNK, 128, H).transpose(1, 0, 2), dtype=np.float32
    )
    mask_h = np.triu(np.ones((128, 128), dtype=np.float32))
    eye_h = np.eye(H, dtype=np.float32)
    ones_h = np.ones((1, H), dtype=np.float32)
    return xh, wqk_h, wv_h, mask_h, eye_h, ones_h


def kernel(x, Wq, Wk, Wv):
    x = np.asarray(x, dtype=np.float32)
    Wq = np.asarray(Wq, dtype=np.float32)
    Wk = np.asarray(Wk, dtype=np.float32)
    Wv = np.asarray(Wv, dtype=np.float32)

    xh, wqk_h, wv_h, mask_h, eye_h, ones_h = _host_inputs(x, Wq, Wk, Wv)

    nc = build_nc(BPC)
    in_maps = [
        {
            "xT": xh[i * BPC : (i + 1) * BPC],
            "wqk": wqk_h,
            "wv": wv_h,
            "mask": mask_h,
            "eye": eye_h,
            "ones": ones_h,
        }
        for i in range(N_CORES)
    ]
    res = run_bass_kernel_spmd(nc, in_maps, list(range(N_CORES)))
    outT = np.concatenate([res.results[i]["outT"] for i in range(N_CORES)], axis=0)
    return np.ascontiguousarray(outT.transpose(0, 2, 1))



# revision 8
# speedup vs baseline: 2.4313x; 2.4313x over previous
"""Single-head causal self-attention on 8 Trainium2 NeuronCores (Bass/Tile).

Problem: x [1024, 256, 384], Wq/Wk/Wv [384, 64] ->
  q,k,v = x@W;  wei = softmax(mask(q k^T / sqrt(384)));  out = wei @ v
Output: [1024, 256, 64] fp32.

v2 design (all-bf16 matmuls, fp32 PSUM accum, host-side normalization):
  - Host pre-transposes x to bf16 xT[b, p, c, t'] = x[b, t, 128c+p] with the
    two t-halves SWAPPED (t' = (t+128) % 256). The same swap applies to the
    s axis anywhere it appears, so "s0" (original s<128) lives in cols
    128:256 of any [*, s'] layout.
  - Per batch (2-batch macro iterations; x DMA'd in 8-batch groups):
      psA  [128,256] = [Wq|Wk]^T x^T  (3-chunk accum; 2 batches share a bank)
      psV  v[s,h] computed directly: stationary xt s-half chunk, moving Wv
      qk_sb (bf16) <- psA via one DVE copy [128, 2, 256]
      weiT: MM-A lhsT=kT_s0 (qk_sb rows 64:128, tile_position trick),
            rhs=q (rows 0:64) N=256 -> psC[:, 0:256]  ([t1|t0] cols)
            MM-B lhsT=kT_s1, rhs=q_t1 N=128 -> psC[:, 256:384]
      P (bf16) = exp(psC * C**-0.5), one ACT instr [128, 384] per batch
      diag-mask multiply on DVE (bf16 4x mode), one instr per 2 batches
      out[t,h] via P-stationary MMs: t1-half = P0_t1^T vaug_s0 + P1^T vaug_s1,
            t0-half = P0_t0^T vaug_s0 (s1 fully masked there); vaug has a
            ones column so col 64/129 of psD = softmax denominators
      out_sb (bf16) <- psD via GpSimd cast; DMA per 2 batches
  - Host divides by denominators and unswaps the t-halves.
"""

import os
from contextlib import ExitStack

import numpy as np
import ml_dtypes

import concourse.bass as bass
import concourse.bacc as bacc
import concourse.tile as tile
from concourse import mybir
from concourse.bass_utils import run_bass_kernel_spmd

N_CORES = 8
B = 1024
T = 256
C = 384
H = 64
BPC = B // N_CORES  # 128 batches per core
NCHUNK = C // 128  # 3
SCALE = float(C) ** -0.5
G8 = 8  # batches per x-load DMA

F32 = mybir.dt.float32
BF16 = mybir.dt.bfloat16
NPBF16 = ml_dtypes.bfloat16


def build_nc(bpc: int = BPC):
    nc = bacc.Bacc(
        "TRN2", target_bir_lowering=False, debug=False, num_devices=N_CORES
    )

    xT = nc.dram_tensor("xT", [bpc, 128, NCHUNK, T], BF16, kind="ExternalInput").ap()
    wqk = nc.dram_tensor("wqk", [128, NCHUNK, 128], BF16, kind="ExternalInput").ap()
    wv = nc.dram_tensor("wv", [128, NCHUNK, H], BF16, kind="ExternalInput").ap()
    mask2 = nc.dram_tensor("mask2", [128, 2, 256], BF16, kind="ExternalInput").ap()
    o = nc.dram_tensor("o", [bpc // 2, 128, 2, 2, 65], BF16, kind="ExternalOutput").ap()

    with ExitStack() as ctx:
        tc = ctx.enter_context(tile.TileContext(nc))

        const = ctx.enter_context(tc.tile_pool(name="const", bufs=1))
        wqk_sb = const.tile([128, NCHUNK, 128], BF16, tag="wqk")
        nc.sync.dma_start(wqk_sb[:], wqk)
        wv_sb = const.tile([128, NCHUNK, H], BF16, tag="wv")
        nc.sync.dma_start(wv_sb[:], wv)
        mask_sb = const.tile([128, 2, 256], BF16, tag="mask")
        nc.sync.dma_start(mask_sb[:], mask2)

        # vaug slots: [128, b(2), s-half(2), 65]; col 64 of each 65-block is a
        # persistent ones column (softmax denominator trick).
        NSLOT = 2
        vaugs = []
        for i in range(NSLOT):
            v_t = const.tile([128, 2, 2, 65], BF16, tag=f"vaug{i}")
            nc.gpsimd.memset(v_t[:, :, :, 64:65], 1.0)
            vaugs.append(v_t)

        xt_pool = ctx.enter_context(tc.tile_pool(name="xt", bufs=2))
        qk_pool = ctx.enter_context(tc.tile_pool(name="qk", bufs=2))
        p_pool = ctx.enter_context(tc.tile_pool(name="p", bufs=2))
        os_pool = ctx.enter_context(tc.tile_pool(name="os", bufs=2))
        k2_pool = ctx.enter_context(tc.tile_pool(name="k2", bufs=3))
        psa_pool = ctx.enter_context(tc.tile_pool(name="psa", bufs=2, space="PSUM"))
        psv_pool = ctx.enter_context(tc.tile_pool(name="psv", bufs=1, space="PSUM"))
        psc_pool = ctx.enter_context(tc.tile_pool(name="psc", bufs=3, space="PSUM"))
        psd_pool = ctx.enter_context(tc.tile_pool(name="psd", bufs=2, space="PSUM"))
        # single persistent bank, two manual slots (bank-granular allocator)
        psv_t = psv_pool.tile([128, 2, 2, 2, H], F32, tag="psv")

        for g8 in range(bpc // G8):
            xt = xt_pool.tile([128, G8, NCHUNK, T], BF16, tag="xt")
            src = xT[g8 * G8 : (g8 + 1) * G8].rearrange("b p c t -> p b c t")
            nc.sync.dma_start(xt[:], src)

            for pair in range(G8 // 2):
                mi = g8 * (G8 // 2) + pair  # macro-iteration index
                vaug = vaugs[mi % NSLOT]
                b0 = pair * 2  # within xt group

                psA = psa_pool.tile([128, 2, T], F32, tag="psa")
                psV = psv_t[:, mi % 2]

                for j in range(2):
                    bb = b0 + j
                    # qk projection: [q|k]^T in [t1|t0] cols
                    for c in range(NCHUNK):
                        nc.tensor.matmul(
                            psA[:, j, :],
                            lhsT=wqk_sb[:, c, :],
                            rhs=xt[:, bb, c, :],
                            start=(c == 0),
                            stop=(c == NCHUNK - 1),
                        )
                    # v direct in [s, h]: stationary xt s'-cols, moving Wv.
                    # s0 (orig s<128) = cols 128:256, s1 = cols 0:128.
                    for s in range(2):
                        scols = slice(128, 256) if s == 0 else slice(0, 128)
                        for c in range(NCHUNK):
                            nc.tensor.matmul(
                                psV[:, j, s, :],
                                lhsT=xt[:, bb, c, scols],
                                rhs=wv_sb[:, c, :],
                                start=(c == 0),
                                stop=(c == NCHUNK - 1),
                            )

                qk_sb = qk_pool.tile([128, 2, T], BF16, tag="qk")
                nc.vector.tensor_copy(qk_sb[:], psA[:])
                # k must sit at SB partition 0 to pair with q in the weiT
                # matmuls (fmap/weights must share a start partition); bounce
                # it via SBUF->SBUF DMA so no compute engine pays for it.
                k2_sb = k2_pool.tile([64, 2, T], BF16, tag="k2")
                nc.gpsimd.dma_start(k2_sb[:], qk_sb[64:128, :, :])

                nc.vector.tensor_copy(vaug[:, :, :, 0:64], psV[:])

                P = p_pool.tile([128, 2, 384], BF16, tag="p")
                for j in range(2):
                    psC = psc_pool.tile([128, 512], F32, tag="psc")
                    # weiT[s', t']: kT_s0 = qk_sb rows 64:128 cols 128:256
                    nc.tensor.matmul(
                        psC[:, 0:256],
                        lhsT=k2_sb[0:64, j, 128:256],
                        rhs=qk_sb[0:64, j, 0:256],
                        start=True,
                        stop=True,
                    )
                    nc.tensor.matmul(
                        psC[:, 256:384],
                        lhsT=k2_sb[0:64, j, 0:128],
                        rhs=qk_sb[0:64, j, 0:128],
                        start=True,
                        stop=True,
                    )
                    nc.scalar.activation(
                        P[:, j, :],
                        psC[:, 0:384],
                        mybir.ActivationFunctionType.Exp,
                        scale=SCALE,
                    )
                # causal diag masks: P cols 128:384 of each batch block
                nc.gpsimd.tensor_mul(P[:, :, 128:384], P[:, :, 128:384], mask_sb[:])

                # padded to one full 2KB PSUM bank so accum groups never
                # cross a bank boundary
                psD = psd_pool.tile([128, 2, 2, 128], F32, tag="psd")
                for j in range(2):
                    # t1-half: s0 contribution + s1 contribution
                    nc.tensor.matmul(
                        psD[:, j, 0, 0:65],
                        lhsT=P[:, j, 0:128],
                        rhs=vaug[:, j, 0, :],
                        start=True,
                        stop=False,
                    )
                    nc.tensor.matmul(
                        psD[:, j, 0, 0:65],
                        lhsT=P[:, j, 256:384],
                        rhs=vaug[:, j, 1, :],
                        start=False,
                        stop=True,
                    )
                    # t0-half: only s0 (s1 fully masked)
                    nc.tensor.matmul(
                        psD[:, j, 1, 0:65],
                        lhsT=P[:, j, 128:256],
                        rhs=vaug[:, j, 0, :],
                        start=True,
                        stop=True,
                    )

                out_sb = os_pool.tile([128, 2, 2, 65], BF16, tag="out")
                # GPSIMD cannot read PSUM; split the cast across ACT and DVE
                nc.scalar.copy(out_sb[:, 0, :, :], psD[:, 0, :, 0:65])
                nc.vector.tensor_copy(out_sb[:, 1, :, :], psD[:, 1, :, 0:65])
                nc.sync.dma_start(o[mi], out_sb[:])

    nc.finalize()
    return nc


def _host_inputs(x, Wq, Wk, Wv):
    B_, T_, C_ = x.shape
    assert (B_, T_, C_) == (B, T, C), (B_, T_, C_)
    # [b, t_hi, t_lo, c, p] -> [b, p, c, t_hi(flipped), t_lo]
    xh = np.ascontiguousarray(
        x.reshape(B, 2, 128, NCHUNK, 128).transpose(0, 4, 3, 1, 2)[:, :, :, ::-1, :]
        .reshape(B, 128, NCHUNK, T)
    ).astype(NPBF16)
    wqk_h = np.ascontiguousarray(
        np.concatenate([Wq, Wk], axis=1).reshape(NCHUNK, 128, 128).transpose(1, 0, 2)
    ).astype(NPBF16)
    wv_h = np.ascontiguousarray(
        Wv.reshape(NCHUNK, 128, H).transpose(1, 0, 2)
    ).astype(NPBF16)
    tri = np.triu(np.ones((128, 128), dtype=np.float32))  # tri[s,t]=1 iff t>=s
    m = np.concatenate([tri, tri], axis=1)  # [128, 256]
    mask_h = np.ascontiguousarray(
        np.broadcast_to(m[:, None, :], (128, 2, 256))
    ).astype(NPBF16)
    return xh, wqk_h, wv_h, mask_h


def _make_in_maps(xh, wqk_h, wv_h, mask_h):
    return [
        {
            "xT": xh[i * BPC : (i + 1) * BPC],
            "wqk": wqk_h,
            "wv": wv_h,
            "mask2": mask_h,
        }
        for i in range(N_CORES)
    ]


def _postprocess(res):
    o = np.concatenate(
        [np.asarray(res.results[i]["o"]) for i in range(N_CORES)], axis=0
    )  # [B//2, 128, 2, 2, 65] bf16
    o = o.astype(np.float32)
    num = o[..., 0:64]  # [B//2, p, j, half, 64]
    den = o[..., 64:65]
    r = num / den  # half 0 -> t = 128+p, half 1 -> t = p
    out = np.empty((B, T, H), dtype=np.float32)
    rb = r.transpose(0, 2, 1, 3, 4).reshape(B, 128, 2, 64)
    out[:, 128:256, :] = rb[:, :, 0, :]
    out[:, 0:128, :] = rb[:, :, 1, :]
    return out


def kernel(x, Wq, Wk, Wv):
    x = np.asarray(x, dtype=np.float32)
    Wq = np.asarray(Wq, dtype=np.float32)
    Wk = np.asarray(Wk, dtype=np.float32)
    Wv = np.asarray(Wv, dtype=np.float32)

    xh, wqk_h, wv_h, mask_h = _host_inputs(x, Wq, Wk, Wv)
    nc = build_nc(BPC)
    in_maps = _make_in_maps(xh, wqk_h, wv_h, mask_h)
    res = run_bass_kernel_spmd(nc, in_maps, list(range(N_CORES)))
    return _postprocess(res)


# revision 9
# speedup vs baseline: 2.7687x; 1.1388x over previous
"""Single-head causal self-attention on 8 Trainium2 NeuronCores (Bass/Tile).

Problem: x [1024, 256, 384], Wq/Wk/Wv [384, 64] ->
  q,k,v = x@W;  wei = softmax(mask(q k^T / sqrt(384)));  out = wei @ v
Output: [1024, 256, 64] fp32.

v2 design (all-bf16 matmuls, fp32 PSUM accum, host-side normalization):
  - Host pre-transposes x to bf16 xT[b, p, c, t'] = x[b, t, 128c+p] with the
    two t-halves SWAPPED (t' = (t+128) % 256). The same swap applies to the
    s axis anywhere it appears, so "s0" (original s<128) lives in cols
    128:256 of any [*, s'] layout.
  - Per batch (2-batch macro iterations; x DMA'd in 8-batch groups):
      psA  [128,256] = [Wq|Wk]^T x^T  (3-chunk accum; 2 batches share a bank)
      psV  v[s,h] computed directly: stationary xt s-half chunk, moving Wv
      qk_sb (bf16) <- psA via one DVE copy [128, 2, 256]
      weiT: MM-A lhsT=kT_s0 (qk_sb rows 64:128, tile_position trick),
            rhs=q (rows 0:64) N=256 -> psC[:, 0:256]  ([t1|t0] cols)
            MM-B lhsT=kT_s1, rhs=q_t1 N=128 -> psC[:, 256:384]
      P (bf16) = exp(psC * C**-0.5), one ACT instr [128, 384] per batch
      diag-mask multiply on DVE (bf16 4x mode), one instr per 2 batches
      out[t,h] via P-stationary MMs: t1-half = P0_t1^T vaug_s0 + P1^T vaug_s1,
            t0-half = P0_t0^T vaug_s0 (s1 fully masked there); vaug has a
            ones column so col 64/129 of psD = softmax denominators
      out_sb (bf16) <- psD via GpSimd cast; DMA per 2 batches
  - Host divides by denominators and unswaps the t-halves.
"""

import os
from contextlib import ExitStack

import numpy as np
import ml_dtypes

import concourse.bass as bass
import concourse.bacc as bacc
import concourse.tile as tile
from concourse import mybir
from concourse.bass_utils import run_bass_kernel_spmd

N_CORES = 8
B = 1024
T = 256
C = 384
H = 64
BPC = B // N_CORES  # 128 batches per core
NCHUNK = C // 128  # 3
SCALE = float(C) ** -0.5
G8 = 8  # batches per x-load DMA

F32 = mybir.dt.float32
BF16 = mybir.dt.bfloat16
NPBF16 = ml_dtypes.bfloat16


def build_nc(bpc: int = BPC):
    nc = bacc.Bacc(
        "TRN2", target_bir_lowering=False, debug=False, num_devices=N_CORES
    )

    xT = nc.dram_tensor("xT", [bpc, 128, NCHUNK, T], BF16, kind="ExternalInput").ap()
    wqk = nc.dram_tensor("wqk", [128, NCHUNK, 128], BF16, kind="ExternalInput").ap()
    wv = nc.dram_tensor("wv", [128, NCHUNK, H], BF16, kind="ExternalInput").ap()
    mask2 = nc.dram_tensor("mask2", [128, 2, 256], BF16, kind="ExternalInput").ap()
    o = nc.dram_tensor("o", [bpc // 2, 128, 2, 2, 65], BF16, kind="ExternalOutput").ap()

    with ExitStack() as ctx:
        tc = ctx.enter_context(tile.TileContext(nc))

        const = ctx.enter_context(tc.tile_pool(name="const", bufs=1))
        wqk_sb = const.tile([128, NCHUNK, 128], BF16, tag="wqk")
        nc.sync.dma_start(wqk_sb[:], wqk)
        wv_sb = const.tile([128, NCHUNK, H], BF16, tag="wv")
        nc.sync.dma_start(wv_sb[:], wv)
        mask_sb = const.tile([128, 2, 256], BF16, tag="mask")
        nc.sync.dma_start(mask_sb[:], mask2)

        # vaug slots: [128, b(2), s-half(2), 65]; col 64 of each 65-block is a
        # persistent ones column (softmax denominator trick).
        NSLOT = 4
        vaugs = []
        for i in range(NSLOT):
            v_t = const.tile([128, 2, 2, 65], BF16, tag=f"vaug{i}")
            nc.gpsimd.memset(v_t[:, :, :, 64:65], 1.0)
            vaugs.append(v_t)

        xt_pool = ctx.enter_context(tc.tile_pool(name="xt", bufs=2))
        qk_pool = ctx.enter_context(tc.tile_pool(name="qk", bufs=3))
        p_pool = ctx.enter_context(tc.tile_pool(name="p", bufs=3))
        os_pool = ctx.enter_context(tc.tile_pool(name="os", bufs=3))
        k2_pool = ctx.enter_context(tc.tile_pool(name="k2", bufs=3))
        psa_pool = ctx.enter_context(tc.tile_pool(name="psa", bufs=2, space="PSUM"))
        psv_pool = ctx.enter_context(tc.tile_pool(name="psv", bufs=1, space="PSUM"))
        psc_pool = ctx.enter_context(tc.tile_pool(name="psc", bufs=3, space="PSUM"))
        psd_pool = ctx.enter_context(tc.tile_pool(name="psd", bufs=2, space="PSUM"))
        # single persistent bank, two manual slots (bank-granular allocator)
        psv_t = psv_pool.tile([128, 2, 2, 2, H], F32, tag="psv")

        for g8 in range(bpc // G8):
            xt = xt_pool.tile([128, G8, NCHUNK, T], BF16, tag="xt")
            src = xT[g8 * G8 : (g8 + 1) * G8].rearrange("b p c t -> p b c t")
            nc.sync.dma_start(xt[:], src)

            for pair in range(G8 // 2):
                mi = g8 * (G8 // 2) + pair  # macro-iteration index
                vaug = vaugs[mi % NSLOT]
                b0 = pair * 2  # within xt group

                psA = psa_pool.tile([128, 2, T], F32, tag="psa")
                psV = psv_t[:, mi % 2]

                # qk projection for both batches at once (N=512)
                for c in range(NCHUNK):
                    nc.tensor.matmul(
                        psA[:, :, :],
                        lhsT=wqk_sb[:, c, :],
                        rhs=xt[:, b0 : b0 + 2, c, :],
                        start=(c == 0),
                        stop=(c == NCHUNK - 1),
                    )
                for j in range(2):
                    bb = b0 + j
                    # v direct in [s, h]: stationary xt s'-cols, moving Wv.
                    # s0 (orig s<128) = cols 128:256, s1 = cols 0:128.
                    for s in range(2):
                        scols = slice(128, 256) if s == 0 else slice(0, 128)
                        for c in range(NCHUNK):
                            nc.tensor.matmul(
                                psV[:, j, s, :],
                                lhsT=xt[:, bb, c, scols],
                                rhs=wv_sb[:, c, :],
                                start=(c == 0),
                                stop=(c == NCHUNK - 1),
                            )

                qk_sb = qk_pool.tile([128, 2, T], BF16, tag="qk")
                nc.vector.tensor_copy(qk_sb[:], psA[:])
                # k must sit at SB partition 0 to pair with q in the weiT
                # matmuls (fmap/weights must share a start partition); bounce
                # it via SBUF->SBUF DMA so no compute engine pays for it.
                k2_sb = k2_pool.tile([64, 2, T], BF16, tag="k2")
                nc.gpsimd.dma_start(k2_sb[:], qk_sb[64:128, :, :])

                nc.vector.tensor_copy(vaug[:, :, :, 0:64], psV[:])

                P = p_pool.tile([128, 2, 384], BF16, tag="p")
                for j in range(2):
                    psC = psc_pool.tile([128, 512], F32, tag="psc")
                    # weiT[s', t']: kT_s0 = qk_sb rows 64:128 cols 128:256
                    nc.tensor.matmul(
                        psC[:, 0:256],
                        lhsT=k2_sb[0:64, j, 128:256],
                        rhs=qk_sb[0:64, j, 0:256],
                        start=True,
                        stop=True,
                    )
                    nc.tensor.matmul(
                        psC[:, 256:384],
                        lhsT=k2_sb[0:64, j, 0:128],
                        rhs=qk_sb[0:64, j, 0:128],
                        start=True,
                        stop=True,
                    )
                    nc.scalar.activation(
                        P[:, j, :],
                        psC[:, 0:384],
                        mybir.ActivationFunctionType.Exp,
                        scale=SCALE,
                    )
                # causal diag masks: P cols 128:384 of each batch block
                nc.vector.tensor_mul(P[:, :, 128:384], P[:, :, 128:384], mask_sb[:])

                # padded to one full 2KB PSUM bank so accum groups never
                # cross a bank boundary
                psD = psd_pool.tile([128, 2, 2, 128], F32, tag="psd")
                for j in range(2):
                    # t1-half: s0 contribution + s1 contribution
                    nc.tensor.matmul(
                        psD[:, j, 0, 0:65],
                        lhsT=P[:, j, 0:128],
                        rhs=vaug[:, j, 0, :],
                        start=True,
                        stop=False,
                    )
                    nc.tensor.matmul(
                        psD[:, j, 0, 0:65],
                        lhsT=P[:, j, 256:384],
                        rhs=vaug[:, j, 1, :],
                        start=False,
                        stop=True,
                    )
                    # t0-half: only s0 (s1 fully masked)
                    nc.tensor.matmul(
                        psD[:, j, 1, 0:65],
                        lhsT=P[:, j, 128:256],
                        rhs=vaug[:, j, 0, :],
                        start=True,
                        stop=True,
                    )

                out_sb = os_pool.tile([128, 2, 2, 65], BF16, tag="out")
                # GPSIMD cannot read PSUM; split the cast across ACT and DVE
                nc.scalar.copy(out_sb[:, 0, :, :], psD[:, 0, :, 0:65])
                nc.vector.tensor_copy(out_sb[:, 1, :, :], psD[:, 1, :, 0:65])
                nc.sync.dma_start(o[mi], out_sb[:])

    nc.finalize()
    return nc


def _host_inputs(x, Wq, Wk, Wv):
    B_, T_, C_ = x.shape
    assert (B_, T_, C_) == (B, T, C), (B_, T_, C_)
    # [b, t_hi, t_lo, c, p] -> [b, p, c, t_hi(flipped), t_lo]
    xh = np.ascontiguousarray(
        x.reshape(B, 2, 128, NCHUNK, 128).transpose(0, 4, 3, 1, 2)[:, :, :, ::-1, :]
        .reshape(B, 128, NCHUNK, T)
    ).astype(NPBF16)
    wqk_h = np.ascontiguousarray(
        np.concatenate([Wq, Wk], axis=1).reshape(NCHUNK, 128, 128).transpose(1, 0, 2)
    ).astype(NPBF16)
    wv_h = np.ascontiguousarray(
        Wv.reshape(NCHUNK, 128, H).transpose(1, 0, 2)
    ).astype(NPBF16)
    tri = np.triu(np.ones((128, 128), dtype=np.float32))  # tri[s,t]=1 iff t>=s
    m = np.concatenate([tri, tri], axis=1)  # [128, 256]
    mask_h = np.ascontiguousarray(
        np.broadcast_to(m[:, None, :], (128, 2, 256))
    ).astype(NPBF16)
    return xh, wqk_h, wv_h, mask_h


def _make_in_maps(xh, wqk_h, wv_h, mask_h):
    return [
        {
            "xT": xh[i * BPC : (i + 1) * BPC],
            "wqk": wqk_h,
            "wv": wv_h,
            "mask2": mask_h,
        }
        for i in range(N_CORES)
    ]


def _postprocess(res):
    o = np.concatenate(
        [np.asarray(res.results[i]["o"]) for i in range(N_CORES)], axis=0
    )  # [B//2, 128, 2, 2, 65] bf16
    o = o.astype(np.float32)
    num = o[..., 0:64]  # [B//2, p, j, half, 64]
    den = o[..., 64:65]
    r = num / den  # half 0 -> t = 128+p, half 1 -> t = p
    out = np.empty((B, T, H), dtype=np.float32)
    rb = r.transpose(0, 2, 1, 3, 4).reshape(B, 128, 2, 64)
    out[:, 128:256, :] = rb[:, :, 0, :]
    out[:, 0:128, :] = rb[:, :, 1, :]
    return out


def kernel(x, Wq, Wk, Wv):
    x = np.asarray(x, dtype=np.float32)
    Wq = np.asarray(Wq, dtype=np.float32)
    Wk = np.asarray(Wk, dtype=np.float32)
    Wv = np.asarray(Wv, dtype=np.float32)

    xh, wqk_h, wv_h, mask_h = _host_inputs(x, Wq, Wk, Wv)
    nc = build_nc(BPC)
    in_maps = _make_in_maps(xh, wqk_h, wv_h, mask_h)
    res = run_bass_kernel_spmd(nc, in_maps, list(range(N_CORES)))
    return _postprocess(res)
